# revision 1
# baseline (speedup 1.0000x reference)
"""ChamferLoss kernel for 8 Trainium2 NeuronCores.

Problem: pred (4,8192,3) f32, gt (4,8192,3) f32 ->
  loss = mean_b[ mean(pred2gt_b) + mean(gt2pred_b) + max(pred2gt_b) ]   (scalar f32)
where pred2gt[b,i] = min_j ||pred[b,i]-gt[b,j]||^2 and gt2pred[b,j] = min_i (same).

Sharding: data-parallel over B (2 cores per batch) x sequence-sharded rows.
Each core computes row-mins of two 4096x8192 distance blocks (dual orientation:
pred-half vs gt-full, and gt-half vs pred-full), so ALL reductions are free-axis
row reductions; the host does only the tiny final mean/max combines.

Distance computation: one K=18 bf16 matmul per tile via the augmented split-
precision form  d = nx + ny - 2 x.y  with x = xh + xl (bf16 hi/lo split) and
norms split into 3 bf16 parts; PSUM accumulates in fp32, so results are
fp32-accurate (abs err ~1e-4, dominated by the bf16-split representation).

Reduction: per 2048-column PSUM unit, ScalarE copies the upper 1024 columns to
SBUF; a custom fused DVE op (min body + min accumulate) then reduces the lower
1024 PSUM columns against the SBUF copy in a single 1x pass, draining PSUM
through both the DVE and ACT read ports concurrently.
"""

from contextlib import ExitStack

import numpy as np
import ml_dtypes

import concourse.bass as bass
import concourse.tile as tile
from concourse import bacc, mybir
from concourse import dve_ops
from concourse.bass_utils import run_bass_kernel_spmd
from concourse.dve_ops import DveOp
from concourse.dve_spec import Spec, Src0, Src1, C0, minn, lower
from concourse.dve_uop import DveOpSpec

B = 4
N = 8192          # pred points per batch
M = 8192          # gt points per batch
NCORES = 8
HALF = 4096       # rows per core per orientation
K = 18            # augmented contraction rows
ITILE = 128       # rows per matmul tile
NSTRIP = 512      # matmul moving free dim
UNIT = 2048       # columns per psum unit (4 matmuls, 4 banks)
NUNITS = M // UNIT          # 4 units per i-tile
NITILES = HALF // ITILE     # 32 i-tiles per orientation
BIG = 3.0e38

_bf16 = ml_dtypes.bfloat16


# --------------------------------------------------------------------------- #
# Custom fused DVE op: out = min(in0, in1); accum_out = min(s0, min_k out)
# --------------------------------------------------------------------------- #

def _ttmin_ref(in0, in1, s0, s1, imm2):
    out = np.minimum(in0.astype(np.float32), in1.astype(np.float32))
    s0v = s0 if np.ndim(s0) == 0 else np.asarray(s0).reshape(-1)
    return out, np.minimum(out.min(axis=-1), s0v)


def _register_min_op() -> DveOp:
    name = "TT_MIN_RED_ANT"
    for o in dve_ops.OPS:
        if o.name == name:
            return o
    spec = Spec(body=minn(Src0, Src1), accum=minn, accum_init=C0, reference=_ttmin_ref)
    shas = {}
    for ver in ("v3", "v4"):
        try:
            s = DveOpSpec(name=name, opcode=0, uops=lower(spec, ver=ver), rd1_en=True)
            shas[ver] = s.sha(ver)
        except Exception:
            pass
    op = DveOp(name, spec, subdim=False, uops_sha=shas)
    dve_ops.OPS.append(op)
    dve_ops._SUB_OPCODE_FOR_NAME[name] = dve_ops._CUSTOM_DVE_ROW_BASE + len(dve_ops.OPS) - 1
    dve_ops.CUSTOM_DVE_SPECS[name] = spec
    return op


# --------------------------------------------------------------------------- #
# Bass program (identical SPMD program on all 8 cores)
# --------------------------------------------------------------------------- #

_CACHE: dict = {}
VARIANT = "pack2d"


def _build_program(reps: int = 1, loop: int = 1, variant: str = "split"):
    """variant: 'split' (ACT copy + custom DVE), 'dve' (plain DVE reduce only)."""
    op = _register_min_op()
    nc = bacc.Bacc("TRN2", target_bir_lowering=False, debug=False, num_devices=NCORES)

    packed = variant.startswith("pack2")
    KP = 32 + K if packed else K  # packed lhsT/rhs carry rows at partitions 0..17 and 32..49
    LW = HALF if variant.startswith("pack2d") else ((HALF // 2) if packed else HALF)
    ins = {}
    outs = {}
    for o in ("E", "F"):
        ins[f"lhsT_{o}"] = nc.dram_tensor(
            f"lhsT_{o}", [KP, LW], mybir.dt.bfloat16, kind="ExternalInput").ap()
        ins[f"rhs_{o}"] = nc.dram_tensor(
            f"rhs_{o}", [KP, M], mybir.dt.bfloat16, kind="ExternalInput").ap()
        outs[o] = nc.dram_tensor(
            f"out{o}", [ITILE, NITILES], mybir.dt.float32, kind="ExternalOutput").ap()

    with tile.TileContext(nc) as tc:
        with ExitStack() as ctx:
            if loop > 1:
                ctx.enter_context(tc.For_i(0, loop, 1))
            inp = ctx.enter_context(tc.tile_pool(name="inp", bufs=2))
            psum = ctx.enter_context(tc.tile_pool(name="psum", bufs=2, space="PSUM"))
            acp = ctx.enter_context(tc.tile_pool(name="acp", bufs=3))
            scr = ctx.enter_context(tc.tile_pool(name="scr", bufs=3))
            stp = ctx.enter_context(tc.tile_pool(name="stp", bufs=3))
            ost = ctx.enter_context(tc.tile_pool(name="ost", bufs=1))

            for o in ("E", "F") * reps:
                lhsT = inp.tile([KP, LW], mybir.dt.bfloat16, tag="lhsT")
                nc.sync.dma_start(out=lhsT[:], in_=ins[f"lhsT_{o}"][:])
                rhs = inp.tile([KP, M], mybir.dt.bfloat16, tag="rhs")
                nc.sync.dma_start(out=rhs[:], in_=ins[f"rhs_{o}"][:])

                if variant.startswith("pack2d"):
                    outstage = ost.tile([ITILE, NITILES], mybir.dt.float32,
                                        tag="outstage")
                    for t in range(NITILES):
                        strip = stp.tile([ITILE, 4], mybir.dt.float32, tag="strip")
                        cp = None
                        for u in range(8):  # 1024-col units (2 strips, one per group)
                            pt = psum.tile([ITILE, 1024], mybir.dt.float32,
                                           tag="pt", bufs=4)
                            for g in range(2):
                                j0 = (2 * u + g) * NSTRIP
                                nc.tensor.matmul(
                                    pt[:, g * NSTRIP:(g + 1) * NSTRIP],
                                    lhsT[32 * g:32 * g + K,
                                         t * ITILE:(t + 1) * ITILE],
                                    rhs[32 * g:32 * g + K, j0:j0 + NSTRIP],
                                    start=True, stop=True)
                            if u % 2 == 0:
                                cp = acp.tile([ITILE, 1024], mybir.dt.float32,
                                              tag="cp")
                                nc.scalar.copy(cp[:], pt[:])
                            else:
                                sc = scr.tile([ITILE, 1024], mybir.dt.bfloat16,
                                              tag="sc")
                                nc.vector._custom_dve(
                                    op, out=sc[:], in0=pt[:], in1=cp[:],
                                    s0=BIG,
                                    accum_out=strip[:, u // 2:u // 2 + 1])
                        nc.vector.tensor_reduce(
                            outstage[:, t:t + 1], strip[:],
                            axis=mybir.AxisListType.X, op=mybir.AluOpType.min)
                    nc.sync.dma_start(out=outs[o][:], in_=outstage[:])
                    continue

                if packed:
                    outstage = ost.tile([ITILE, NITILES], mybir.dt.float32,
                                        tag="outstage")
                    NS = NITILES // 2  # 16 supertiles of 2 i-tiles
                    for s in range(NS):
                        strip = stp.tile([ITILE, 2, 4], mybir.dt.float32, tag="strip")
                        cp = None
                        for u in range(8):  # units of 2 j-strips
                            pt = psum.tile([ITILE, 2048], mybir.dt.float32, tag="pt")
                            for jj in range(2):
                                for g in range(2):
                                    j0 = (u * 2 + jj) * NSTRIP
                                    nc.tensor.matmul(
                                        pt[:, g * 1024 + jj * NSTRIP:
                                           g * 1024 + (jj + 1) * NSTRIP],
                                        lhsT[32 * g:32 * g + K,
                                             s * ITILE:(s + 1) * ITILE],
                                        rhs[32 * g:32 * g + K, j0:j0 + NSTRIP],
                                        start=True, stop=True)
                            if variant == "pack2_cheap":
                                nc.vector.tensor_reduce(
                                    strip[:, u % 2, u // 2:u // 2 + 1],
                                    pt[:, 0:64],
                                    axis=mybir.AxisListType.X,
                                    op=mybir.AluOpType.min)
                                continue
                            if variant == "pack2_mm":
                                continue
                            if u % 2 == 0:
                                cp = acp.tile([ITILE, 2048], mybir.dt.float32,
                                              tag="cp")
                                nc.scalar.copy(cp[:], pt[:])
                            else:
                                for g in range(2):
                                    sc = scr.tile([ITILE, 1024], mybir.dt.bfloat16,
                                                  tag="sc")
                                    nc.vector._custom_dve(
                                        op, out=sc[:],
                                        in0=pt[:, g * 1024:(g + 1) * 1024],
                                        in1=cp[:, g * 1024:(g + 1) * 1024],
                                        s0=BIG,
                                        accum_out=strip[:, g, u // 2:u // 2 + 1])
                        if variant == "pack2_mm":
                            nc.vector.memset(outstage[:, 2 * s:2 * s + 2], 0.0)
                        else:
                            for g in range(2):
                                nc.vector.tensor_reduce(
                                    outstage[:, 2 * s + g:2 * s + g + 1],
                                    strip[:, g, :],
                                    axis=mybir.AxisListType.X, op=mybir.AluOpType.min)
                    nc.sync.dma_start(out=outs[o][:], in_=outstage[:])
                    continue

                outstage = ost.tile([ITILE, NITILES], mybir.dt.float32, tag="outstage")
                if variant in ("mm", "mm_act"):
                    nc.vector.memset(outstage[:], 0.0)
                if variant == "mm_dvec":
                    cp0 = acp.tile([ITILE, UNIT // 2], mybir.dt.float32, tag="cp0")
                    nc.vector.memset(cp0[:], 0.0)
                for t in range(NITILES):
                    w = lhsT[:, t * ITILE:(t + 1) * ITILE]
                    strip = stp.tile([ITILE, NUNITS], mybir.dt.float32, tag="strip")
                    for u in range(NUNITS):
                        pt = psum.tile([ITILE, UNIT], mybir.dt.float32, tag="pt")
                        for k in range(UNIT // NSTRIP):
                            j0 = u * UNIT + k * NSTRIP
                            nc.tensor.matmul(
                                pt[:, k * NSTRIP:(k + 1) * NSTRIP],
                                w, rhs[:, j0:j0 + NSTRIP],
                                start=True, stop=True)
                        if variant == "split":
                            cp = acp.tile([ITILE, UNIT // 2], mybir.dt.float32, tag="cp")
                            nc.scalar.copy(cp[:], pt[:, UNIT // 2:UNIT])
                            sc = scr.tile([ITILE, UNIT // 2], mybir.dt.bfloat16, tag="sc")
                            nc.vector._custom_dve(
                                op, out=sc[:], in0=pt[:, 0:UNIT // 2], in1=cp[:],
                                s0=BIG, accum_out=strip[:, u:u + 1])
                        elif variant == "dve":
                            nc.vector.tensor_reduce(
                                strip[:, u:u + 1], pt[:],
                                axis=mybir.AxisListType.X, op=mybir.AluOpType.min)
                        elif variant == "cheap":
                            nc.vector.tensor_reduce(
                                strip[:, u:u + 1], pt[:, 0:64],
                                axis=mybir.AxisListType.X, op=mybir.AluOpType.min)
                        elif variant == "mm":
                            pass  # PE only
                        elif variant == "mm_act":
                            cp = acp.tile([ITILE, UNIT // 2], mybir.dt.float32, tag="cp")
                            nc.scalar.copy(cp[:], pt[:, UNIT // 2:UNIT])
                        elif variant == "mm_dvec":
                            sc = scr.tile([ITILE, UNIT // 2], mybir.dt.bfloat16, tag="sc")
                            nc.vector._custom_dve(
                                op, out=sc[:], in0=pt[:, 0:UNIT // 2], in1=cp0[:],
                                s0=BIG, accum_out=strip[:, u:u + 1])
                    if variant in ("split", "dve", "mm_dvec", "cheap"):
                        nc.vector.tensor_reduce(
                            outstage[:, t:t + 1], strip[:],
                            axis=mybir.AxisListType.X, op=mybir.AluOpType.min)
                nc.sync.dma_start(out=outs[o][:], in_=outstage[:])

    nc.compile()
    return nc


# --------------------------------------------------------------------------- #
# Host-side input prep: augmented split-precision matrices
# --------------------------------------------------------------------------- #

def _split3(v):
    """Split fp32/fp64 array into 3 bf16 parts summing to ~v."""
    a = v.astype(_bf16).astype(np.float64)
    r = v - a
    b = r.astype(np.float32).astype(_bf16).astype(np.float64)
    c = (r - b).astype(np.float32).astype(_bf16).astype(np.float64)
    return a, b, c


def _augment(xrows, ycols):
    """Build (lhsT [K, nx], rhs [K, ny]) bf16 so that lhsT.T @ rhs [i,j]
    ~= ||x_i - y_j||^2 in fp32 precision.  xrows (nx,3), ycols (ny,3) f32."""
    nx_, ny_ = xrows.shape[0], ycols.shape[0]
    xh = xrows.astype(_bf16).astype(np.float64)
    xl32 = (xrows.astype(np.float64) - xh).astype(np.float32)
    xl = xl32.astype(_bf16).astype(np.float64)
    yh = ycols.astype(_bf16).astype(np.float64)
    yl32 = (ycols.astype(np.float64) - yh).astype(np.float32)
    yl = yl32.astype(_bf16).astype(np.float64)

    xe = xh + xl          # effective points (exactly representable as bf16+bf16)
    ye = yh + yl
    nxv = (xe * xe).sum(1)
    nyv = (ye * ye).sum(1)
    nxa, nxb, nxc = _split3(nxv)
    nya, nyb, nyc = _split3(nyv)

    lhsT = np.zeros((K, nx_), np.float32)
    rhs = np.zeros((K, ny_), np.float32)
    lhsT[0:3] = xh.T; rhs[0:3] = -2.0 * yh.T
    lhsT[3:6] = xh.T; rhs[3:6] = -2.0 * yl.T
    lhsT[6:9] = xl.T; rhs[6:9] = -2.0 * yh.T
    lhsT[9:12] = xl.T; rhs[9:12] = -2.0 * yl.T
    lhsT[12] = nxa; rhs[12] = 1.0
    lhsT[13] = nxb; rhs[13] = 1.0
    lhsT[14] = nxc; rhs[14] = 1.0
    lhsT[15] = 1.0; rhs[15] = nya
    lhsT[16] = 1.0; rhs[16] = nyb
    lhsT[17] = 1.0; rhs[17] = nyc
    return lhsT.astype(_bf16), rhs.astype(_bf16)


def _pack2d(lhsT, rhs):
    """Duplicate all K rows into PE row groups 0 and 32 (same i-tile both groups)."""
    KP = 32 + K
    lp = np.zeros((KP, HALF), np.float32).astype(_bf16)
    lp[0:K] = lhsT
    lp[32:32 + K] = lhsT
    rp = np.zeros((KP, M), np.float32).astype(_bf16)
    rp[0:K] = rhs
    rp[32:32 + K] = rhs
    return lp, rp


def _pack2(lhsT, rhs):
    """Interleave pairs of i-tiles into PE row groups 0 and 32."""
    KP = 32 + K
    lp = np.zeros((KP, HALF // 2), np.float32).astype(_bf16)
    v = np.asarray(lhsT).reshape(K, NITILES // 2, 2, ITILE)
    lp[0:K] = v[:, :, 0, :].reshape(K, HALF // 2)
    lp[32:32 + K] = v[:, :, 1, :].reshape(K, HALF // 2)
    rp = np.zeros((KP, M), np.float32).astype(_bf16)
    rp[0:K] = rhs
    rp[32:32 + K] = rhs
    return lp, rp


def _make_in_maps(pred, gt, variant="split"):
    in_maps = []
    rhs_gt = {}
    rhs_pred = {}
    for b in range(B):
        # rhs matrices are shared by the two cores of a batch; build once
        _, rhs_gt[b] = _augment(pred[b][:1], gt[b])
        _, rhs_pred[b] = _augment(gt[b][:1], pred[b])
    for c in range(NCORES):
        b, h = c // 2, c % 2
        rows = slice(h * HALF, (h + 1) * HALF)
        lhsT_E, _ = _augment(pred[b][rows], gt[b][:1])
        lhsT_F, _ = _augment(gt[b][rows], pred[b][:1])
        rE, rF = rhs_gt[b], rhs_pred[b]
        if variant.startswith("pack2d"):
            lhsT_E, rE = _pack2d(lhsT_E, rE)
            lhsT_F, rF = _pack2d(lhsT_F, rF)
        elif variant.startswith("pack2"):
            lhsT_E, rE = _pack2(lhsT_E, rE)
            lhsT_F, rF = _pack2(lhsT_F, rF)
        in_maps.append({
            "lhsT_E": lhsT_E, "rhs_E": rE,
            "lhsT_F": lhsT_F, "rhs_F": rF,
        })
    return in_maps


def _unstage(arr):
    """[128, 32] staging -> [4096] vector with row index t*128+p."""
    return np.asarray(arr, np.float32).T.reshape(-1)


def kernel(pred, gt):
    pred = np.asarray(pred, dtype=np.float32)
    gt = np.asarray(gt, dtype=np.float32)
    assert pred.shape == (B, N, 3) and gt.shape == (B, M, 3)

    if "nc" not in _CACHE:
        _CACHE["nc"] = _build_program(variant=VARIANT)
    nc = _CACHE["nc"]

    in_maps = _make_in_maps(pred, gt, variant=VARIANT)
    res = run_bass_kernel_spmd(nc, in_maps, list(range(NCORES)))

    loss_terms = []
    for b in range(B):
        p2g = np.concatenate([_unstage(res.results[2 * b]["outE"]),
                              _unstage(res.results[2 * b + 1]["outE"])])
        g2p = np.concatenate([_unstage(res.results[2 * b]["outF"]),
                              _unstage(res.results[2 * b + 1]["outF"])])
        loss_terms.append(p2g.mean(dtype=np.float64)
                          + g2p.mean(dtype=np.float64)
                          + np.float64(p2g.max()))
    return np.float32(np.mean(loss_terms))



# revision 4
# speedup vs baseline: 7.0009x; 7.0009x over previous
"""ChamferLoss kernel for 8 Trainium2 NeuronCores.

Problem: pred (4,8192,3) f32, gt (4,8192,3) f32 ->
  loss = mean_b[ mean(pred2gt_b) + mean(gt2pred_b) + max(pred2gt_b) ]   (scalar f32)
where pred2gt[b,i] = min_j ||pred[b,i]-gt[b,j]||^2 and gt2pred[b,j] = min_i (same).

Sharding: one full orientation per core (8 = 4 batches x 2 orientations).
Core 2b computes pred->gt for batch b (8192 x 8192 distances), core 2b+1
computes gt->pred.  Each core receives two compact bf16 "slabs" (9 x 8192):
[uh; ul; mn] where uh/ul is the bf16 hi/lo split of sqrt(2)*points^T and mn is
a 3-way bf16 split of -||x~||^2.  We compute the NEGATED distance
  d' = 2 x.y - ||x||^2 - ||y||^2  = -||x-y||^2
so the same slab serves as lhs (rows) and rhs (columns) on different cores
(no -2 scaling asymmetry), and all reductions become max instead of min.

On device, lhsT [50, 8192] and rhs [50, 8192] bf16 are assembled purely with
DMA row placement (K=18 used rows duplicated into PE row groups 0 and 32) +
two memsets for the broadcast-ones rows; PSUM accumulates the K=18 matmul in
fp32.  Per 1024-column PSUM pair, ScalarE copies the even unit to SBUF; a
custom fused DVE op (max body + max accumulate) reduces the odd PSUM unit
against the copy in one pass.  Device output per core is (128, 2) f32:
[row-chunk sum of row-max', row-chunk min of row-max']; the host finishes the
tiny 128-way mean/max combines and flips signs.

Execution: the jitted shard_map callable is built ONCE and cached — the
stock run_bass_kernel_spmd path re-traces and re-lowers jax.jit on every
call, which costs ~500 ms/call through the axon tunnel.  Wire traffic per
call is 2.4 MB of slabs + 8 KB of outputs (vs 19.7 MB for host-built
matrices).
"""

import math
import numpy as np
import ml_dtypes

import jax
from jax.sharding import Mesh, PartitionSpec
from jax.experimental.shard_map import shard_map

import concourse.bass as bass
import concourse.tile as tile
from concourse import bacc, mybir
from concourse import dve_ops
from concourse.dve_ops import DveOp
from concourse.dve_spec import Spec, Src0, Src1, C0, maxx, lower
from concourse.dve_uop import DveOpSpec
from concourse.bass2jax import (
    _bass_exec_p,
    install_neuronx_cc_hook,
    partition_id_tensor,
)

B = 4
N = 8192          # pred points per batch
M = 8192          # gt points per batch
NCORES = 8
ROWS = 8192       # lhs rows per core (full orientation)
K = 18            # augmented contraction rows
ITILE = 128       # rows per matmul tile
NSTRIP = 512      # matmul moving free dim
NITILES = ROWS // ITILE     # 64 i-tiles
KP = 50           # packed partition extent (row groups at 0 and 32)
BIG = 3.0e38
SQRT2 = math.sqrt(2.0)

_bf16 = ml_dtypes.bfloat16


# --------------------------------------------------------------------------- #
# Custom fused DVE op: out = max(in0, in1); accum_out = max(s0, max_k out)
# --------------------------------------------------------------------------- #

def _ttmax_ref(in0, in1, s0, s1, imm2):
    out = np.maximum(in0.astype(np.float32), in1.astype(np.float32))
    s0v = s0 if np.ndim(s0) == 0 else np.asarray(s0).reshape(-1)
    return out, np.maximum(out.max(axis=-1), s0v)


def _register_max_op() -> DveOp:
    name = "TT_MAX_RED_ANT"
    for o in dve_ops.OPS:
        if o.name == name:
            return o
    spec = Spec(body=maxx(Src0, Src1), accum=maxx, accum_init=C0,
                reference=_ttmax_ref)
    shas = {}
    for ver in ("v3", "v4"):
        try:
            s = DveOpSpec(name=name, opcode=0, uops=lower(spec, ver=ver),
                          rd1_en=True)
            shas[ver] = s.sha(ver)
        except Exception:
            pass
    op = DveOp(name, spec, subdim=False, uops_sha=shas)
    dve_ops.OPS.append(op)
    dve_ops._SUB_OPCODE_FOR_NAME[name] = \
        dve_ops._CUSTOM_DVE_ROW_BASE + len(dve_ops.OPS) - 1
    dve_ops.CUSTOM_DVE_SPECS[name] = spec
    return op


# --------------------------------------------------------------------------- #
# Bass program (identical SPMD program on all 8 cores)
# --------------------------------------------------------------------------- #

_CACHE: dict = {}


def _build_program():
    op = _register_max_op()
    nc = bacc.Bacc("TRN2", target_bir_lowering=False, debug=False,
                   num_devices=NCORES)

    L = nc.dram_tensor("L", [9, ROWS], mybir.dt.bfloat16,
                       kind="ExternalInput").ap()
    R = nc.dram_tensor("R", [9, M], mybir.dt.bfloat16,
                       kind="ExternalInput").ap()
    out = nc.dram_tensor("out", [ITILE, 2], mybir.dt.float32,
                         kind="ExternalOutput").ap()

    with tile.TileContext(nc) as tc:
        with tc.tile_pool(name="mat", bufs=1) as mat, \
             tc.tile_pool(name="psum", bufs=2, space="PSUM") as psum, \
             tc.tile_pool(name="acp", bufs=3) as acp, \
             tc.tile_pool(name="scr", bufs=3) as scr, \
             tc.tile_pool(name="stp", bufs=3) as stp, \
             tc.tile_pool(name="ost", bufs=1) as ost:

            lhsT = mat.tile([64, ROWS], mybir.dt.bfloat16, tag="lhsT")
            rhs = mat.tile([64, M], mybir.dt.bfloat16, tag="rhs")

            # Engine ops must start at partition 0/32: memset whole tiles to
            # 1.0 (the broadcast-ones rows), then DMA data rows over them.
            # lhsT: uh->{0:3,3:6,+32}, ul->{6:9,9:12,+32}, mn->{12:15,+32},
            # ones at {15:18,+32} left from memset.
            nc.vector.memset(lhsT[:], 1.0)
            nc.vector.memset(rhs[:], 1.0)
            for g in (0, 32):
                nc.sync.dma_start(out=lhsT[g + 0:g + 3, :], in_=L[0:3, :])
                nc.sync.dma_start(out=lhsT[g + 3:g + 6, :], in_=L[0:3, :])
                nc.sync.dma_start(out=lhsT[g + 6:g + 9, :], in_=L[3:6, :])
                nc.sync.dma_start(out=lhsT[g + 9:g + 12, :], in_=L[3:6, :])
                nc.sync.dma_start(out=lhsT[g + 12:g + 15, :], in_=L[6:9, :])
            # rhs: wh->{0:3,6:9,+32}, wl->{3:6,9:12,+32}, ones {12:15,+32}
            # from memset, mn->{15:18,+32}.
            for g in (0, 32):
                nc.sync.dma_start(out=rhs[g + 0:g + 3, :], in_=R[0:3, :])
                nc.sync.dma_start(out=rhs[g + 3:g + 6, :], in_=R[3:6, :])
                nc.sync.dma_start(out=rhs[g + 6:g + 9, :], in_=R[0:3, :])
                nc.sync.dma_start(out=rhs[g + 9:g + 12, :], in_=R[3:6, :])
                nc.sync.dma_start(out=rhs[g + 15:g + 18, :], in_=R[6:9, :])

            outstage = ost.tile([ITILE, NITILES], mybir.dt.float32,
                                tag="outstage")
            for t in range(NITILES):
                strip = stp.tile([ITILE, 4], mybir.dt.float32, tag="strip")
                cp = None
                for u in range(8):  # 1024-col units (2 strips, one per group)
                    pt = psum.tile([ITILE, 1024], mybir.dt.float32,
                                   tag="pt", bufs=4)
                    for g in range(2):
                        j0 = (2 * u + g) * NSTRIP
                        nc.tensor.matmul(
                            pt[:, g * NSTRIP:(g + 1) * NSTRIP],
                            lhsT[32 * g:32 * g + K,
                                 t * ITILE:(t + 1) * ITILE],
                            rhs[32 * g:32 * g + K, j0:j0 + NSTRIP],
                            start=True, stop=True)
                    if u % 2 == 0:
                        cp = acp.tile([ITILE, 1024], mybir.dt.float32,
                                      tag="cp")
                        nc.scalar.copy(cp[:], pt[:])
                    else:
                        sc = scr.tile([ITILE, 1024], mybir.dt.bfloat16,
                                      tag="sc")
                        nc.vector._custom_dve(
                            op, out=sc[:], in0=pt[:], in1=cp[:],
                            s0=-BIG,
                            accum_out=strip[:, u // 2:u // 2 + 1])
                nc.vector.tensor_reduce(
                    outstage[:, t:t + 1], strip[:],
                    axis=mybir.AxisListType.X, op=mybir.AluOpType.max)

            outf = ost.tile([ITILE, 2], mybir.dt.float32, tag="outf")
            nc.vector.tensor_reduce(
                outf[:, 0:1], outstage[:],
                axis=mybir.AxisListType.X, op=mybir.AluOpType.add)
            nc.vector.tensor_reduce(
                outf[:, 1:2], outstage[:],
                axis=mybir.AxisListType.X, op=mybir.AluOpType.min)
            nc.sync.dma_start(out=out[:], in_=outf[:])

    nc.compile()
    return nc


# --------------------------------------------------------------------------- #
# Cached jitted SPMD runner (avoids per-call jit re-trace + re-lower)
# --------------------------------------------------------------------------- #

def _build_runner(nc, n_cores):
    install_neuronx_cc_hook()
    partition_name = (nc.partition_id_tensor.name
                      if nc.partition_id_tensor else None)

    in_names, out_names, out_avals, out_shapes = [], [], [], []
    for alloc in nc.m.functions[0].allocations:
        if not isinstance(alloc, mybir.MemoryLocationSet):
            continue
        name = alloc.memorylocations[0].name
        if alloc.kind == "ExternalInput":
            if name != partition_name:
                in_names.append(name)
        elif alloc.kind == "ExternalOutput":
            shape = tuple(alloc.tensor_shape)
            dtype = mybir.dt.np(alloc.dtype)
            out_names.append(name)
            out_avals.append(jax.core.ShapedArray(shape, dtype))
            out_shapes.append((shape, dtype))
    n_params = len(in_names)
    n_outs = len(out_avals)
    all_in_names = list(in_names) + list(out_names)
    if partition_name is not None:
        all_in_names.append(partition_name)

    donate = tuple(range(n_params, n_params + n_outs))

    def _body(*args):
        operands = list(args)
        if partition_name is not None:
            operands.append(partition_id_tensor())
        outs = _bass_exec_p.bind(
            *operands,
            out_avals=tuple(out_avals),
            in_names=tuple(all_in_names),
            out_names=tuple(out_names),
            lowering_input_output_aliases=(),
            sim_require_finite=True,
            sim_require_nnan=True,
            nc=nc,
        )
        return tuple(outs)

    devices = jax.devices()[:n_cores]
    mesh = Mesh(np.asarray(devices), ("core",))
    in_specs = (PartitionSpec("core"),) * (n_params + n_outs)
    out_specs = (PartitionSpec("core"),) * n_outs
    sharded = jax.jit(
        shard_map(_body, mesh=mesh, in_specs=in_specs, out_specs=out_specs,
                  check_rep=False),
        donate_argnums=donate, keep_unused=True,
    )

    def run(in_maps):
        concat_in = [
            np.concatenate([np.asarray(in_maps[c][name])
                            for c in range(n_cores)], axis=0)
            for name in in_names
        ]
        concat_zeros = [
            np.zeros((n_cores * s[0], *s[1:]), d) for (s, d) in out_shapes
        ]
        out_arrs = sharded(*concat_in, *concat_zeros)
        return [
            {name: np.asarray(out_arrs[i]).reshape(
                n_cores, *out_shapes[i][0])[c]
             for i, name in enumerate(out_names)}
            for c in range(n_cores)
        ]

    return run


# --------------------------------------------------------------------------- #
# Host-side input prep: compact bf16 slabs
# --------------------------------------------------------------------------- #

def _slab(points):
    """points (P,3) f32 -> (9,P) bf16 slab [uh; ul; split3(-|x~|^2)]
    with u = sqrt(2)*points^T."""
    u = (SQRT2 * points.T).astype(np.float32)          # (3,P)
    uh = u.astype(_bf16)
    uh32 = uh.astype(np.float32)
    ul = (u - uh32).astype(_bf16)
    ue = uh32.astype(np.float64) + ul.astype(np.float64)
    m = -0.5 * np.sum(ue * ue, axis=0)                  # -|x~|^2  (f64)
    a = m.astype(np.float32).astype(_bf16)
    r = m - a.astype(np.float64)
    b = r.astype(np.float32).astype(_bf16)
    c = (r - b.astype(np.float64)).astype(np.float32).astype(_bf16)
    out = np.empty((9, points.shape[0]), _bf16)
    out[0:3] = uh
    out[3:6] = ul
    out[6] = a
    out[7] = b
    out[8] = c
    return out


def _make_in_maps(pred, gt):
    in_maps = []
    for b in range(B):
        sp = _slab(pred[b])
        sg = _slab(gt[b])
        in_maps.append({"L": sp, "R": sg})   # core 2b:   pred -> gt
        in_maps.append({"L": sg, "R": sp})   # core 2b+1: gt -> pred
    return in_maps


def kernel(pred, gt):
    pred = np.asarray(pred, dtype=np.float32)
    gt = np.asarray(gt, dtype=np.float32)
    assert pred.shape == (B, N, 3) and gt.shape == (B, M, 3)

    if "run" not in _CACHE:
        nc = _build_program()
        _CACHE["run"] = _build_runner(nc, NCORES)
    run = _CACHE["run"]

    results = run(_make_in_maps(pred, gt))

    loss_terms = []
    for b in range(B):
        oE = results[2 * b]["out"]        # pred->gt maxd' stats
        oF = results[2 * b + 1]["out"]    # gt->pred
        mean_p2g = -float(oE[:, 0].sum(dtype=np.float64)) / N
        mean_g2p = -float(oF[:, 0].sum(dtype=np.float64)) / M
        max_p2g = -float(oE[:, 1].min())
        loss_terms.append(mean_p2g + mean_g2p + max_p2g)
    return np.float32(np.mean(loss_terms))
